# revision 8
# baseline (speedup 1.0000x reference)
"""Trainium2 Bass kernel for nn_APNRRU (complex-rotation RNN scan).

Strategy (pure data parallelism, batch 4096 -> 512 per core):
  Host (numpy): FIR front-end, phase normalizers r_t, per-step frame
  rotations rho_t = r_{t+1} * conj(r_t) PRE-BROADCAST to the full
  128-partition pattern, and the final output de-rotation.
  Device (Bass, 8 cores SPMD): the sequential 1024-step scan in the
  rotated frame.

Device layout (per core): batch 512 -> 2 independent groups x 3 blocks
x 86 columns (free dim).  Each group's state packs 105 partitions:
rows 0:48 = I-parts (3 blocks x 16), rows 64:112 = Q-parts, rows
112:121 = hA.  The complex swap is a partition-offset read (+-64).

v3 design notes (instruction-count + chain-latency bound):
  - Every engine instruction costs ~130-360ns fixed; the recurrence
    chain is ~8 instructions.  The two groups are SOFTWARE-PIPELINED
    at a half-step skew (emit P1(g0,t), P2(g1,t-1), sigma(t), P2(g0,t),
    P1(g1,t)) so PE/Act run one group's matmul+tanh phase while
    DVE/GpSimd run the other group's gate+rotation phase.  In-order
    engines head-of-line block, so ISSUE order must match readiness.
  - State and w are fp16 (validated: rel err 0.0033 vs 0.0034 fp32;
    bf16 fails the gate).  fp32 matmuls lower to TWO HW instructions
    (778ns on-chain) - fp16 keeps every matmul a single 1-cyc/row op.
  - rho patterns come pre-broadcast from HBM (DMA is otherwise idle).
  - Input-side u-matmul (no recurrence dep) batched: one PE inst
    covers 2 steps x 2 groups; state matmul accumulates into PSUM.
  - sigmoid merged across groups: state lives as halves of one
    [128,172] tile, one Act inst per step instead of two.
  - w = z*t2 + sg fused as one scalar_tensor_tensor (z is [128,1]).
  - Output projection batched over 4 steps ([12,344] fp16 matmul),
    staged PSUM->SBUF by one DVE cast, DMA'd out per chunk.
"""

import os
import sys

import numpy as np
from ml_dtypes import bfloat16

sys.path.insert(0, "/opt/trn_rl_repo")

B, S, H, HA, WIN, NF, HN = 4096, 1024, 16, 3, 16, 3, 16
NCORES = 8
BL = B // NCORES          # 512 batch per core
G = 2                     # groups per core
NB = 3                    # blocks per group
FD = 86                   # free dim (batch columns per block); 2*3*86=516>=512
CH = 32                   # timesteps per input DMA chunk
W4 = 4                    # steps per output-projection batch

_GRAPH_CACHE = {}
LAST_RESULT = None
LAST_RUN_WALL = None


def _host_frontend(x, fir_I_w, fir_Q_w):
    xI = np.ascontiguousarray(x[..., 0], np.float32)
    xQ = np.ascontiguousarray(x[..., 1], np.float32)
    mag = np.sqrt(xI * xI + xQ * xQ)
    rr = xI / mag
    ri = -xQ / mag

    b = x.shape[0]
    pad = np.zeros((b, WIN - 1), np.float32)
    pI = np.concatenate([pad, xI], axis=1)
    pQ = np.concatenate([pad, xQ], axis=1)
    swv = np.lib.stride_tricks.sliding_window_view
    wI = swv(pI, WIN, axis=1)          # [B,S,WIN]
    wQ = swv(pQ, WIN, axis=1)
    fiw = fir_I_w.astype(np.float32)
    fqw = fir_Q_w.astype(np.float32)
    fII = wI @ fiw.T                   # [B,S,NF]
    fQQ = wQ @ fqw.T
    fQI = wI @ fqw.T
    fIQ = wQ @ fiw.T
    I_fir = fII - fQQ
    Q_fir = fQI + fIQ
    I4 = np.concatenate([I_fir, xI[..., None]], axis=-1)   # [B,S,4]
    Q4 = np.concatenate([Q_fir, xQ[..., None]], axis=-1)
    In = rr[..., None] * I4 - ri[..., None] * Q4
    Qn = ri[..., None] * I4 + rr[..., None] * Q4
    IQ = np.stack([In, Qn], axis=-1).reshape(b, S, 2 * (NF + 1))  # [B,S,8]
    return IQ, rr, ri


def _host_rho(rr, ri):
    # rho[:, t] = r_{t+1} * conj(r_t); last step gets identity.
    rho_r = np.ones((rr.shape[0], S), np.float32)
    rho_i = np.zeros((rr.shape[0], S), np.float32)
    rho_r[:, :-1] = rr[:, 1:] * rr[:, :-1] + ri[:, 1:] * ri[:, :-1]
    rho_i[:, :-1] = ri[:, 1:] * rr[:, :-1] - rr[:, 1:] * ri[:, :-1]
    return rho_r, rho_i


def _arow(k, j):
    return 16 * k + j


def _brow(k, j):
    return 64 + 16 * k + j


def _hrow(k, j):
    return 112 + 3 * k + j


def _make_consts(W_u_w, W_u_b, W_h_w, W_h_b, Z, out_I_w, out_Q_w):
    Wu = np.asarray(W_u_w, np.float32)      # [16, 43]
    Wh = np.asarray(W_h_w, np.float32)      # [35, 16]
    wI = np.asarray(out_I_w, np.float32)[0]  # [16]
    wQ = np.asarray(out_Q_w, np.float32)[0]
    z = np.asarray(Z, np.float32)[0]         # [35]

    wu_iq = np.zeros((24, 48), np.float32)
    wu_st = np.zeros((128, 48), np.float32)
    wh105 = np.zeros((48, 128), np.float32)
    bu48 = np.zeros((48, 1), np.float32)
    bh105 = np.zeros((128, 1), np.float32)
    z105 = np.zeros((128, 1), np.float32)
    wo105 = np.zeros((128, 12), np.float32)

    for k in range(NB):
        for i in range(16):
            col = 16 * k + i
            bu48[col, 0] = W_u_b[i]
            for j in range(8):
                wu_iq[8 * k + j, col] = Wu[i, j]
            for j in range(16):
                wu_st[_arow(k, j), col] = Wu[i, 8 + j]
                wu_st[_brow(k, j), col] = Wu[i, 24 + j]
            for j in range(3):
                wu_st[_hrow(k, j), col] = Wu[i, 40 + j]
        for j in range(16):
            ra, rb = _arow(k, j), _brow(k, j)
            bh105[ra, 0] = W_h_b[j]
            bh105[rb, 0] = W_h_b[16 + j]
            z105[ra, 0] = z[j]
            z105[rb, 0] = z[16 + j]
            for i in range(16):
                wh105[16 * k + i, ra] = Wh[j, i]
                wh105[16 * k + i, rb] = Wh[16 + j, i]
            wo105[ra, 4 * k + 0] = wI[j]
            wo105[rb, 4 * k + 0] = -wQ[j]
            wo105[ra, 4 * k + 1] = wQ[j]
            wo105[rb, 4 * k + 1] = wI[j]
            wo105[ra, 4 * k + 2] = wI[j]
            wo105[rb, 4 * k + 2] = wQ[j]
            wo105[ra, 4 * k + 3] = -wQ[j]
            wo105[rb, 4 * k + 3] = wI[j]
        for j in range(3):
            rh = _hrow(k, j)
            bh105[rh, 0] = W_h_b[32 + j]
            z105[rh, 0] = z[32 + j]
            for i in range(16):
                wh105[16 * k + i, rh] = Wh[32 + j, i]

    return {
        "wu_iq": wu_iq.astype(bfloat16),
        "wu_st": wu_st.astype(np.float16),    # fp16: state matmul
        "wh": wh105.astype(bfloat16),
        "bu": bu48,
        "bh": bh105,
        "z": z105,
        "wo": wo105.astype(np.float16),       # fp16: w is fp16
    }


def _build_graph(c_val, steps):
    from concourse import bacc, mybir, tile

    assert steps % W4 == 0 and CH % W4 == 0 and steps >= W4

    nc = bacc.Bacc()
    f32 = mybir.dt.float32
    f16 = mybir.dt.float16
    bf16 = mybir.dt.bfloat16

    iqa_d = nc.dram_tensor("iqa", [24, steps * 2 * FD], bf16,
                           kind="ExternalInput")
    rho_d = nc.dram_tensor("rho", [128 * G, steps * 2 * FD], bf16,
                           kind="ExternalInput")
    wu_iq_d = nc.dram_tensor("wu_iq", [24, 48], bf16, kind="ExternalInput")
    wu_st_d = nc.dram_tensor("wu_st", [128, 48], f16, kind="ExternalInput")
    wh_d = nc.dram_tensor("wh", [48, 128], bf16, kind="ExternalInput")
    bu_d = nc.dram_tensor("bu", [48, 1], f32, kind="ExternalInput")
    bh_d = nc.dram_tensor("bh", [128, 1], f32, kind="ExternalInput")
    z_d = nc.dram_tensor("z", [128, 1], f32, kind="ExternalInput")
    wo_d = nc.dram_tensor("wo", [128, 12], f16, kind="ExternalInput")
    out_d = nc.dram_tensor("out", [12 * G, steps * FD], bf16,
                           kind="ExternalOutput")

    ch = min(CH, steps)
    MUL = mybir.AluOpType.mult
    ADD = mybir.AluOpType.add
    TANH = mybir.ActivationFunctionType.Tanh
    SIG = mybir.ActivationFunctionType.Sigmoid

    with tile.TileContext(nc) as tc:
        with (
            tc.tile_pool(name="consts", bufs=1) as cpool,
            tc.tile_pool(name="chunks", bufs=2) as chpool,
            tc.tile_pool(name="work", bufs=2) as wpool,
            tc.tile_pool(name="w4p", bufs=2) as w4pool,
            tc.tile_pool(name="state", bufs=2) as gpool,
            tc.tile_pool(name="ostage", bufs=2) as opool,
            tc.tile_pool(name="pspu", bufs=2, space="PSUM") as pupool,
            tc.tile_pool(name="pswh", bufs=2, space="PSUM") as whpool,
            tc.tile_pool(name="pswo", bufs=2, space="PSUM") as wopool,
        ):
            wu_iq_s = cpool.tile([24, 48], bf16, tag="wu_iq")
            wu_st_s = cpool.tile([128, 48], f16, tag="wu_st")
            wh_s = cpool.tile([48, 128], bf16, tag="wh")
            bu_s = cpool.tile([48, 1], f32, tag="bu")
            bh_s = cpool.tile([128, 1], f32, tag="bh")
            z_s = cpool.tile([128, 1], f32, tag="z")
            wo_s = cpool.tile([128, 12], f16, tag="wo")
            nc.sync.dma_start(wu_iq_s[:], wu_iq_d[:])
            nc.sync.dma_start(wu_st_s[:], wu_st_d[:])
            nc.sync.dma_start(wh_s[:], wh_d[:])
            nc.sync.dma_start(bu_s[:], bu_d[:])
            nc.sync.dma_start(bh_s[:], bh_d[:])
            nc.sync.dma_start(z_s[:], z_d[:])
            nc.sync.dma_start(wo_s[:], wo_d[:])

            # state for both groups lives in one [128, 2*FD] fp16 tile so
            # the sigmoid is a single Act instruction per step.
            st_prev = gpool.tile([128, 2 * FD], f16, tag="st", name="st")
            nc.vector.memset(st_prev[:], 0.0)
            st_cur = None

            # PE warm-up: one dummy matmul per const used as stationary.
            warm = whpool.tile([128, FD], f32, tag="mmw")
            for ct in (wu_iq_s, wh_s, wu_st_s, wo_s):
                m = min(ct.shape[1], 128)
                nc.tensor.matmul(warm[0:m, 0:1], ct[:, 0:m], ct[:, 0:1],
                                 start=True, stop=True)

            iq_ch = None
            rho_ch = [{} for _ in range(G)]   # chunk_idx -> tile
            ostg = [{} for _ in range(G)]
            w4 = [None] * G
            pu = {}                           # (t//2) -> psum tile
            v1 = [None] * G
            t2 = [None] * G
            sgm = None
            states = {}                       # t -> state tile

            def p1_phase(g, t):
                nonlocal sgm
                ci = t // ch
                if t % 2 == 0 and g == 0:
                    pu.pop(t // 2 - 2, None)
                    pu[t // 2] = pupool.tile([48, 4 * FD], f32, tag="pu",
                                             name="pu")
                    o2 = ((t % ch) // 2) * 4 * FD
                    nc.tensor.matmul(pu[t // 2][:, 0:4 * FD], wu_iq_s[:],
                                     iq_ch[:, o2:o2 + 4 * FD],
                                     start=True, stop=False)
                put = pu[t // 2]
                sc = (2 * (t % 2) + g) * FD
                sprev = states[t - 1]
                nc.tensor.matmul(put[:, sc:sc + FD], wu_st_s[:],
                                 sprev[:, g * FD:(g + 1) * FD],
                                 start=False, stop=True,
                                 skip_group_check=True)
                v1[g] = wpool.tile([48, FD], bf16, tag=f"v1{g}",
                                   name=f"v1{g}")
                nc.scalar.activation(v1[g][:], put[0:48, sc:sc + FD],
                                     TANH, bias=bu_s[:])
                mm2 = whpool.tile([128, FD], f32, tag="wh2", name=f"wh2{g}")
                nc.tensor.matmul(mm2[:], wh_s[:], v1[g][:],
                                 start=True, stop=True)
                t2[g] = wpool.tile([128, FD], f32, tag=f"t2{g}",
                                   name=f"t2{g}")
                nc.scalar.activation(t2[g][:], mm2[:], TANH, bias=bh_s[:])

            def p2_phase(g, t):
                ci = t // ch
                if t % W4 == 0:
                    w4[g] = w4pool.tile([128, W4 * FD], f16,
                                        tag=f"w4{g}", name=f"w4{g}")
                ws = w4[g][:, (t % W4) * FD:(t % W4 + 1) * FD]
                nc.vector.scalar_tensor_tensor(
                    ws, t2[g][:], z_s[:, 0:1],
                    sgm[:, g * FD:(g + 1) * FD], MUL, ADD)
                roff = (t % ch) * 2 * FD
                rch = rho_ch[g][ci]
                p1 = wpool.tile([128, FD], f32, tag=f"p1{g}", name=f"p1{g}")
                nc.vector.tensor_mul(p1[:], rch[:, roff:roff + FD], ws)
                p2 = wpool.tile([128, FD], f32, tag=f"p2{g}", name=f"p2{g}")
                nc.gpsimd.tensor_mul(
                    p2[0:64, :], rch[64:128, roff + FD:roff + 2 * FD],
                    w4[g][64:128, (t % W4) * FD:(t % W4 + 1) * FD])
                nc.gpsimd.tensor_mul(
                    p2[64:128, :], rch[0:64, roff + FD:roff + 2 * FD],
                    w4[g][0:64, (t % W4) * FD:(t % W4 + 1) * FD])
                # g0's add on DVE, g1's on GpSimd (engine balance)
                dst = states[t][:, g * FD:(g + 1) * FD]
                if g == 0:
                    nc.vector.tensor_add(dst, p1[:], p2[:])
                else:
                    nc.gpsimd.tensor_add(dst, p1[:], p2[:])
                if t % W4 == W4 - 1:
                    wop = wopool.tile([12, W4 * FD], f32, tag="wo",
                                      name=f"wo{g}")
                    nc.tensor.matmul(wop[:], wo_s[:], w4[g][:],
                                     start=True, stop=True)
                    t0 = (t - (W4 - 1)) % ch
                    nc.vector.tensor_copy(
                        ostg[g][ci][:, t0 * FD:(t0 + W4) * FD], wop[:])
                if (t + 1) % ch == 0 or t == steps - 1:
                    t0 = (t // ch) * ch
                    n = (t + 1 - t0) * FD
                    nc.sync.dma_start(
                        out_d[12 * g:12 * (g + 1), t0 * FD:t0 * FD + n],
                        ostg[g][ci][:, 0:n])

            states[-1] = st_prev
            for t in range(steps):
                ci = t // ch
                if t % ch == 0:
                    iq_ch = chpool.tile([24, ch * 2 * FD], bf16, tag="iq",
                                        name="iq")
                    nc.sync.dma_start(
                        iq_ch[:], iqa_d[:, t * 2 * FD:(t + ch) * 2 * FD])
                    for g in range(G):
                        rho_ch[g][ci] = chpool.tile(
                            [128, ch * 2 * FD], bf16, tag=f"rho{g}",
                            name=f"rho{g}")
                        nc.sync.dma_start(
                            rho_ch[g][ci][:],
                            rho_d[128 * g:128 * (g + 1),
                                  t * 2 * FD:(t + ch) * 2 * FD])
                        ostg[g][ci] = opool.tile([12, ch * FD], bf16,
                                                 tag=f"ost{g}",
                                                 name=f"ost{g}")
                        rho_ch[g].pop(ci - 2, None)
                        ostg[g].pop(ci - 2, None)

                p1_phase(0, t)
                if t > 0:
                    p2_phase(1, t - 1)
                # merged sigmoid over both groups' previous state
                sgm_new = wpool.tile([128, 2 * FD], f32, tag="sgm",
                                     name="sgm")
                nc.scalar.activation(sgm_new[:], states[t - 1][:], SIG,
                                     scale=float(c_val))
                sgm = sgm_new
                states[t] = gpool.tile([128, 2 * FD], f16, tag="st",
                                       name="st")
                p2_phase(0, t)
                p1_phase(1, t)
                states.pop(t - 2, None)
            p2_phase(1, steps - 1)
    nc.compile()
    return nc


def _pack_core(IQ, rho_r, rho_i, c0, steps):
    """Assemble per-core input arrays: interleaved IQ for the batched
    input matmul and fully pre-broadcast rho rotation patterns."""
    iqa = np.zeros((24, steps * 2 * FD), bfloat16)
    rho = np.zeros((128 * G, steps * 2 * FD), bfloat16)
    iq_v = iqa.reshape(24, steps, 2, FD)          # [row, t, g, col]
    rho_v = rho.reshape(G, 128, steps, 2, FD)     # [g, row, t, rr/ri, col]
    for g in range(G):
        rho_v[g, 112:121, :, 0, :] = 1.0
        for k in range(NB):
            lo = c0 + 258 * g + 86 * k
            hi = min(lo + FD, c0 + BL)
            nb = hi - lo
            iq_v[8 * k:8 * k + 8, :, g, :nb] = (
                IQ[lo:hi, :steps].transpose(2, 1, 0).astype(bfloat16))
            rr_blk = np.ones((steps, FD), np.float32)
            rr_blk[:, :nb] = rho_r[lo:hi, :steps].T
            ri_blk = np.zeros((steps, FD), np.float32)
            ri_blk[:, :nb] = rho_i[lo:hi, :steps].T
            rr_b = rr_blk.astype(bfloat16)
            ri_b = ri_blk.astype(bfloat16)
            rho_v[g, 16 * k:16 * k + 16, :, 0, :] = rr_b[None]
            rho_v[g, 64 + 16 * k:64 + 16 * k + 16, :, 0, :] = rr_b[None]
            rho_v[g, 16 * k:16 * k + 16, :, 1, :] = ri_b[None]
            rho_v[g, 64 + 16 * k:64 + 16 * k + 16, :, 1, :] = -ri_b[None]
    return iqa, rho


def kernel(**inputs):
    x = np.asarray(inputs["x"], np.float32)
    fir_I_w = np.asarray(inputs["fir_I_w"], np.float32)
    fir_Q_w = np.asarray(inputs["fir_Q_w"], np.float32)
    W_u_w = np.asarray(inputs["W_u_w"], np.float32)
    W_u_b = np.asarray(inputs["W_u_b"], np.float32)
    W_h_w = np.asarray(inputs["W_h_w"], np.float32)
    W_h_b = np.asarray(inputs["W_h_b"], np.float32)
    C = np.asarray(inputs["C"], np.float32)
    Z = np.asarray(inputs["Z"], np.float32)
    out_I_w = np.asarray(inputs["out_I_w"], np.float32)
    out_Q_w = np.asarray(inputs["out_Q_w"], np.float32)

    steps = int(os.environ.get("BASS_STEPS", S))

    IQ, rr, ri = _host_frontend(x, fir_I_w, fir_Q_w)
    rho_r, rho_i = _host_rho(rr, ri)

    consts = _make_consts(W_u_w, W_u_b, W_h_w, W_h_b, Z, out_I_w, out_Q_w)

    key = (float(C[0]), steps)
    if key not in _GRAPH_CACHE:
        _GRAPH_CACHE[key] = _build_graph(float(C[0]), steps)
    nc = _GRAPH_CACHE[key]

    in_maps = []
    for c in range(NCORES):
        iqa, rho = _pack_core(IQ, rho_r, rho_i, c * BL, steps)
        in_maps.append({**consts, "iqa": iqa, "rho": rho})

    from concourse.bass_utils import run_bass_kernel_spmd

    import time as _time
    _t0 = _time.time()
    trace = bool(os.environ.get("BASS_TRACE"))
    try:
        res = run_bass_kernel_spmd(nc, in_maps, core_ids=list(range(NCORES)),
                                   trace=trace)
    except ModuleNotFoundError:
        res = run_bass_kernel_spmd(nc, in_maps, core_ids=list(range(NCORES)))
    global LAST_RESULT, LAST_RUN_WALL
    LAST_RUN_WALL = _time.time() - _t0
    LAST_RESULT = res

    out = np.empty((B, steps, 2), np.float32)
    for c in range(NCORES):
        o = res.results[c]["out"].astype(np.float32)     # [24, steps*FD]
        o = o.reshape(12 * G, steps, FD)
        for g in range(G):
            for k in range(NB):
                lo = c * BL + 258 * g + 86 * k
                hi = min(lo + FD, (c + 1) * BL)
                nb = hi - lo
                pp = o[12 * g + 4 * k + 0, :, :nb].T       # [nb, steps]
                qq = o[12 * g + 4 * k + 1, :, :nb].T
                p2 = o[12 * g + 4 * k + 2, :, :nb].T
                q2 = o[12 * g + 4 * k + 3, :, :nb].T
                rrs = rr[lo:hi, :steps]
                ris = ri[lo:hi, :steps]
                out[lo:hi, :, 0] = rrs * pp + ris * qq
                out[lo:hi, :, 1] = rrs * p2 + ris * q2
    return out
